# revision 16
# baseline (speedup 1.0000x reference)
"""Trainium2 Bass kernel for the CantorExpert MoE-routing expert.

Contract: kernel(**inputs) takes FULL unsharded numpy inputs and returns the
FULL output tuple (Q, K, V, K_aff, Q_aff, mask), matching reference().

Strategy:
  - Host: routing (fingerprint mask) + gather of the expert's 512-wide
    feature band, transpose to feature-major [512, N], zero-pad, shard
    tokens x output-columns (TOK_SHARDS x COL_SHARDS) across 8 NeuronCores.
  - Device (per core, feature-major layout, zero on-device transposes):
      gate:  H = W1^T-stationary matmuls -> PSUM [128hid, T]
             exact GELU via Erf (shares the ACT table set with Sigmoid)
             gate_w2 matmul with column-replicated weights -> the per-token
             gate logit lands broadcast across all 128 partitions
      qkv:   W_all = [wq^T | wk^T | wv^T | (dirs@wq)^T | (dirs@wk)^T]
             (pentachoron affinities folded in as 10 extra output columns)
             computed from UNSCALED X; the per-token gate scale is applied
             during the PSUM->SBUF eviction as a fused DVE tensor_mul.
      float32r matmuls (full PE rate at free-dim>=256, ~1e-4 rel err).
      Input DMAs split (x | gate weights | wall column-chunks) on the SWDGE
      ring so compute overlaps loading; output DMAs ride the HWDGE ring.
  - Host: reassemble [3082, N] -> Q,K,V [B,p,1024], K_aff/Q_aff [5,B,p].
"""

import contextlib
import math

import numpy as np

import concourse.bass as bass
import concourse.bacc as bacc
import concourse.mybir as mybir
from concourse import tile
from concourse.bass_utils import run_bass_kernel_spmd

# ---- problem constants (hardcoded per contract) ----
EXPERT_ID = 3
NUM_EXPERTS = 8
FULL_DIM = 4096
EXPERT_DIM = 1024
SLICE = FULL_DIM // NUM_EXPERTS          # 512
S0 = EXPERT_ID * SLICE                   # 1536
S1 = S0 + SLICE                          # 2048
FP_MIN = EXPERT_ID / NUM_EXPERTS         # 0.375
FP_MAX = (EXPERT_ID + 1) / NUM_EXPERTS   # 0.5
GATE_HID = SLICE // 4                    # 128
N_CORES = 8
NAFF = 10                                # 5 q-aff + 5 k-aff columns
M_COLS = 3 * EXPERT_DIM + NAFF           # 3082
KT = SLICE // 128                        # 4 contraction tiles

FP32 = mybir.dt.float32
F32R = mybir.dt.float32r

# sharding: token shards x column shards (product must be N_CORES)
TOK_SHARDS = 2
COL_SHARDS = 4

USE_F32R = True       # float32r matmuls: 4x faster than fp32 on the PE
VARIANT = "full"      # full | noout | dmaonly  (component profiling)
MCHUNK_TILES = 2      # wall DMA chunk granularity, in 128-col m-tiles
INV_SQRT2 = 0.7071067811865476

_nc_cache: dict = {}


def _col_ranges():
    """Split the M_COLS output columns into COL_SHARDS ranges on 128-multiples."""
    full_tiles = M_COLS // 128            # 24
    per = int(round(full_tiles / COL_SHARDS))
    bounds = [0]
    for s in range(COL_SHARDS - 1):
        bounds.append(min(full_tiles, (s + 1) * per) * 128)
    bounds.append(M_COLS)
    return [(bounds[i], bounds[i + 1]) for i in range(COL_SHARDS)]


def _mchunks(ncols):
    """Group the ncols output columns into DMA chunks of MCHUNK_TILES m-tiles.
    Returns [(c_off, c_cols, [(m_off_in_chunk, mt), ...]), ...]."""
    tiles = []
    m0 = 0
    while m0 < ncols:
        mt = min(128, ncols - m0)
        tiles.append((m0, mt))
        m0 += mt
    chunks = []
    for i in range(0, len(tiles), MCHUNK_TILES):
        grp = tiles[i:i + MCHUNK_TILES]
        c_off = grp[0][0]
        c_cols = sum(mt for _, mt in grp)
        chunks.append((c_off, c_cols, [(m0 - c_off, mt) for (m0, mt) in grp]))
    return chunks


def _prog_key(T, ncols, aw, b2):
    return (T, ncols, aw, b2, USE_F32R, VARIANT, MCHUNK_TILES,
            TOK_SHARDS, COL_SHARDS)


def prepare(inputs):
    """Host-side routing/sharding. Returns (nc, in_maps, meta)."""
    tokens = np.asarray(inputs["tokens"], dtype=np.float32)
    fingerprints = np.asarray(inputs["fingerprints"], dtype=np.float32)
    alpha = np.float32(np.asarray(inputs["alpha"], dtype=np.float32))
    gate_w1 = np.asarray(inputs["gate_w1"], dtype=np.float32)
    gate_b1 = np.asarray(inputs["gate_b1"], dtype=np.float32)
    gate_w2 = np.asarray(inputs["gate_w2"], dtype=np.float32)
    gate_b2 = np.asarray(inputs["gate_b2"], dtype=np.float32)
    wq = np.asarray(inputs["wq"], dtype=np.float32)
    wk = np.asarray(inputs["wk"], dtype=np.float32)
    wv = np.asarray(inputs["wv"], dtype=np.float32)
    penta = np.asarray(inputs["pentachoron"], dtype=np.float32)

    B = tokens.shape[0]
    mask = (fingerprints >= np.float32(FP_MIN)) & (fingerprints < np.float32(FP_MAX))
    idx = np.flatnonzero(mask)
    p = int(idx.size)

    if p == 0:
        return None, None, {"B": B, "p": 0, "mask": mask}

    dirs = penta / np.linalg.norm(penta, axis=-1, keepdims=True)
    dirs = dirs.astype(np.float32)

    # gather + flatten tokens: X [N, 512], feature-major X^T [512, N]
    Xg = tokens[:, idx, S0:S1]                    # [B, p, 512]
    N = B * p
    X2 = Xg.reshape(N, SLICE)
    T = int(math.ceil(N / (TOK_SHARDS * 128))) * 128
    XTfull = np.zeros((SLICE, TOK_SHARDS * T), np.float32)
    XTfull[:, :N] = X2.T

    def pm(a):
        """[512, S] -> partition-major k-interleaved [128, KT*S]."""
        S = a.shape[1]
        return np.ascontiguousarray(
            a.reshape(KT, 128, S).transpose(1, 0, 2).reshape(128, KT * S)
        )

    w1t = pm(gate_w1.T)                            # [128, KT*128]
    b1 = gate_b1.reshape(128, 1)
    bvec = np.ascontiguousarray(
        np.concatenate([b1, b1 * np.float32(INV_SQRT2)], axis=1)
    ).astype(np.float32)
    w2r = np.ascontiguousarray(
        np.repeat(0.5 * gate_w2.reshape(GATE_HID, 1), 128, axis=1)
    ).astype(np.float32)
    aw = float(1.0 / (1.0 + np.exp(-np.float64(alpha))))
    b2 = float(gate_b2.reshape(-1)[0])

    dq = dirs @ wq                                 # [5, 512]
    dk = dirs @ wk
    wall_full = np.concatenate([wq.T, wk.T, wv.T, dq.T, dk.T], axis=1)  # [512, 3082]

    cranges = _col_ranges()
    ncols_prog = max(c1 - c0 for (c0, c1) in cranges)
    mchunks = _mchunks(ncols_prog)

    tchunks = []
    t0 = 0
    while t0 < T:
        tchunks.append((t0, min(512, T - t0)))
        t0 += 512

    in_maps = []
    core_meta = []
    for tshard in range(TOK_SHARDS):
        base = tshard * T
        xs = np.concatenate(
            [pm(XTfull[:, base + t0:base + t0 + tsz]) for (t0, tsz) in tchunks],
            axis=1,
        )
        for (c0, c1) in cranges:
            wall_c = np.zeros((SLICE, ncols_prog), np.float32)
            wall_c[:, : c1 - c0] = wall_full[:, c0:c1]
            # chunk-major wall: for each m-chunk, a k-interleaved block
            blocks = [pm(wall_c[:, co:co + cc]) for (co, cc, _) in mchunks]
            inbuf = np.concatenate([xs, w1t, w2r] + blocks, axis=1)
            in_maps.append({"inbuf": np.ascontiguousarray(inbuf), "bvec": bvec})
            core_meta.append((tshard, c0, c1))

    key = _prog_key(T, ncols_prog, aw, b2)
    if key not in _nc_cache:
        _nc_cache[key] = _build_prog(T, ncols_prog, aw, b2)
    nc = _nc_cache[key]

    meta = {"B": B, "p": p, "N": N, "T": T, "mask": mask, "core_meta": core_meta,
            "prog_args": (T, ncols_prog, aw, b2)}
    return nc, in_maps, meta


def postprocess(results, meta):
    B, p, mask = meta["B"], meta["p"], meta["mask"]
    if p == 0:
        z = np.zeros((B, 0, EXPERT_DIM), np.float32)
        za = np.zeros((5, B, 0), np.float32)
        return (z, z.copy(), z.copy(), za, za.copy(), mask)
    N, T, core_meta = meta["N"], meta["T"], meta["core_meta"]

    O = np.zeros((M_COLS, TOK_SHARDS * T), np.float32)
    for i, (tshard, c0, c1) in enumerate(core_meta):
        O[c0:c1, tshard * T:(tshard + 1) * T] = results[i]["out"][: c1 - c0, :]
    O = O[:, :N]

    D = EXPERT_DIM
    Q = np.ascontiguousarray(O[0:D].T).reshape(B, p, D)
    K = np.ascontiguousarray(O[D:2 * D].T).reshape(B, p, D)
    V = np.ascontiguousarray(O[2 * D:3 * D].T).reshape(B, p, D)
    Q_aff = np.ascontiguousarray(O[3 * D:3 * D + 5]).reshape(5, B, p)
    K_aff = np.ascontiguousarray(O[3 * D + 5:3 * D + 10]).reshape(5, B, p)

    return (Q, K, V, K_aff, Q_aff, mask)


def run(inputs, trace=False):
    """Full pipeline. Returns ((Q, K, V, K_aff, Q_aff, mask), bass_results)."""
    nc, in_maps, meta = prepare(inputs)
    if nc is None:
        return postprocess(None, meta), None
    res = run_bass_kernel_spmd(nc, in_maps, list(range(N_CORES)), trace=trace)
    return postprocess(res.results, meta), res


def _build_prog(T, ncols, aw, b2, reps=1):
    nc = bacc.Bacc()
    built = _build_body(nc, T, ncols, aw, b2, reps)
    built.finalize()
    return built


def _build_body(nc, T, ncols, aw, b2, reps=1):
    mchunks = _mchunks(ncols)
    X_COLS = KT * T
    G_COLS = KT * GATE_HID + 128
    W_COLS = KT * ncols
    ALL = X_COLS + G_COLS + W_COLS
    OFF_X = 0
    OFF_W1 = X_COLS
    OFF_WALL = OFF_W1 + G_COLS
    # per-chunk column offsets within the wall region (chunk-major layout)
    chunk_off = {}
    o = OFF_WALL
    for (co, cc, _) in mchunks:
        chunk_off[co] = o
        o += KT * cc

    MM_DT_IN = F32R if USE_F32R else FP32
    inbuf = nc.dram_tensor("inbuf", [128, ALL], MM_DT_IN, kind="ExternalInput")
    bvec = nc.dram_tensor("bvec", [128, 2], FP32, kind="ExternalInput")
    out = nc.dram_tensor("out", [ncols, T], FP32, kind="ExternalOutput")

    chunks = []
    t0 = 0
    while t0 < T:
        tsz = min(512, T - t0)
        chunks.append((t0, tsz))
        t0 += tsz

    Erf = mybir.ActivationFunctionType.Erf
    Sigmoid = mybir.ActivationFunctionType.Sigmoid
    mul_op = mybir.AluOpType.mult
    add_op = mybir.AluOpType.add
    MM_DT = F32R if USE_F32R else FP32

    with tile.TileContext(nc) as tc:
        with (
            tc.tile_pool(name="weights", bufs=2) as wpool,
            tc.tile_pool(name="gate_sb", bufs=2) as gpool,
            tc.tile_pool(name="gate_ps", bufs=2, space="PSUM") as gpsum,
            tc.tile_pool(name="out_ps", bufs=4, space="PSUM") as opsum,
            tc.tile_pool(name="out_sb", bufs=4) as opool,
        ):
            def rep_ctx():
                if reps < 0:
                    return tc.For_i(0, -reps, 1)
                return contextlib.nullcontext()

            for _rep in range(max(1, reps if reps > 0 else 1)):
                with rep_ctx():
                    # -- input DMAs: x on the SP HWDGE ring (first in FIFO),
                    #    weights on the SWDGE ring, so both stream in parallel --
                    x_sb = {}
                    xoff = 0
                    for (t0, tsz) in chunks:
                        xt_tile = wpool.tile([128, KT * tsz], MM_DT, tag=f"x{t0}")
                        nc.sync.dma_start(
                            xt_tile[:], inbuf[:, OFF_X + xoff:OFF_X + xoff + KT * tsz]
                        )
                        x_sb[t0] = xt_tile
                        xoff += KT * tsz
                    g_sb = wpool.tile([128, G_COLS], MM_DT, tag="gw")
                    nc.gpsimd.dma_start(g_sb[:], inbuf[:, OFF_W1:OFF_W1 + G_COLS])
                    wall_sb = {}
                    for (co, cc, _) in mchunks:
                        wtile = wpool.tile([128, KT * cc], MM_DT, tag=f"wall{co}")
                        nc.gpsimd.dma_start(
                            wtile[:], inbuf[:, chunk_off[co]:chunk_off[co] + KT * cc]
                        )
                        wall_sb[co] = wtile
                    b_sb = wpool.tile([128, 2], FP32, tag="bvec")
                    nc.sync.dma_start(b_sb[:], bvec[:])

                    if VARIANT == "dmaonly":
                        o_sb = opool.tile([128, 8], FP32, tag="o")
                        nc.vector.tensor_copy(o_sb[:, 0:8], x_sb[0][:, 0:8])
                        nc.sync.dma_start(out[0:128, 0:8], o_sb[:, 0:8])
                        continue

                    def xsl(k, t0, tsz):
                        return x_sb[t0][:, k * tsz:k * tsz + tsz]

                    def w1s(k):
                        return g_sb[:, k * GATE_HID:(k + 1) * GATE_HID]

                    w2s = g_sb[:, KT * GATE_HID:KT * GATE_HID + 128]

                    for (t0, tsz) in chunks:
                        # ---- gate MLP ----
                        ps_h = gpsum.tile([128, tsz], FP32, tag="ps_h")
                        for k in range(KT):
                            nc.tensor.matmul(
                                ps_h[:], w1s(k), xsl(k, t0, tsz),
                                start=(k == 0), stop=(k == KT - 1),
                            )
                        erf_sb = gpool.tile([128, tsz], FP32, tag="erf")
                        nc.scalar.activation(
                            erf_sb[:], ps_h[:], Erf,
                            bias=b_sb[:, 1:2], scale=INV_SQRT2,
                        )
                        xb_sb = gpool.tile([128, tsz], FP32, tag="xb")
                        nc.vector.tensor_scalar_add(xb_sb[:], ps_h[:], b_sb[:, 0:1])
                        # h2 = (erf + 1) * xb  (= 2*gelu(x); 0.5 folded into w2r)
                        h2_sb = gpool.tile([128, tsz], MM_DT, tag="h2")
                        nc.vector.scalar_tensor_tensor(
                            h2_sb[:], erf_sb[:], 1.0, xb_sb[:], add_op, mul_op
                        )
                        ps_g = gpsum.tile([128, tsz], FP32, tag="ps_g")
                        nc.tensor.matmul(ps_g[:], w2s, h2_sb[:], start=True, stop=True)
                        sig_sb = gpool.tile([128, tsz], FP32, tag="sig")
                        nc.scalar.activation(sig_sb[:], ps_g[:], Sigmoid, bias=b2)
                        g_gate = gpool.tile([128, tsz], FP32, tag="g")
                        nc.vector.tensor_scalar(
                            g_gate[:], sig_sb[:], aw, 1.0 - aw, mul_op, add_op
                        )

                        # ---- QKV + folded affinities ----
                        for (co, cc, tiles) in mchunks:
                            wtile = wall_sb[co]
                            for (mo, mt) in tiles:
                                ps_o = opsum.tile([128, tsz], FP32, tag="ps_o")
                                for k in range(KT):
                                    nc.tensor.matmul(
                                        ps_o[:mt, :],
                                        wtile[:, k * cc + mo:k * cc + mo + mt],
                                        xsl(k, t0, tsz),
                                        start=(k == 0), stop=(k == KT - 1),
                                    )
                                o_sb = opool.tile([128, tsz], FP32, tag="o")
                                nc.vector.tensor_mul(
                                    o_sb[:mt, :], ps_o[:mt, :], g_gate[:mt, :]
                                )
                                if VARIANT != "noout":
                                    nc.scalar.dma_start(
                                        out[co + mo:co + mo + mt, t0:t0 + tsz],
                                        o_sb[:mt, :],
                                    )
    return nc


def kernel(**inputs):
    outputs, _ = run(inputs, trace=False)
    return outputs


# revision 17
# speedup vs baseline: 1.0905x; 1.0905x over previous
"""Trainium2 Bass kernel for the CantorExpert MoE-routing expert.

Contract: kernel(**inputs) takes FULL unsharded numpy inputs and returns the
FULL output tuple (Q, K, V, K_aff, Q_aff, mask), matching reference().

Strategy:
  - Host: routing (fingerprint mask) + gather of the expert's 512-wide
    feature band, transpose to feature-major [512, N], zero-pad, shard
    tokens x output-columns (TOK_SHARDS x COL_SHARDS) across 8 NeuronCores.
  - Device (per core, feature-major layout, zero on-device transposes):
      gate:  H = W1^T-stationary matmuls -> PSUM [128hid, T]
             exact GELU via Erf (shares the ACT table set with Sigmoid)
             gate_w2 matmul with column-replicated weights -> the per-token
             gate logit lands broadcast across all 128 partitions
      qkv:   W_all = [wq^T | wk^T | wv^T | (dirs@wq)^T | (dirs@wk)^T]
             (pentachoron affinities folded in as 10 extra output columns)
             computed from UNSCALED X; the per-token gate scale is applied
             during the PSUM->SBUF eviction as a fused DVE tensor_mul.
      float32r matmuls (full PE rate at free-dim>=256, ~1e-4 rel err).
      Input DMAs split (x | gate weights | wall column-chunks) on the SWDGE
      ring so compute overlaps loading; output DMAs ride the HWDGE ring.
  - Host: reassemble [3082, N] -> Q,K,V [B,p,1024], K_aff/Q_aff [5,B,p].
"""

import contextlib
import math

import numpy as np

import concourse.bass as bass
import concourse.bacc as bacc
import concourse.mybir as mybir
from concourse import tile
from concourse.bass_utils import run_bass_kernel_spmd

# ---- problem constants (hardcoded per contract) ----
EXPERT_ID = 3
NUM_EXPERTS = 8
FULL_DIM = 4096
EXPERT_DIM = 1024
SLICE = FULL_DIM // NUM_EXPERTS          # 512
S0 = EXPERT_ID * SLICE                   # 1536
S1 = S0 + SLICE                          # 2048
FP_MIN = EXPERT_ID / NUM_EXPERTS         # 0.375
FP_MAX = (EXPERT_ID + 1) / NUM_EXPERTS   # 0.5
GATE_HID = SLICE // 4                    # 128
N_CORES = 8
NAFF = 10                                # 5 q-aff + 5 k-aff columns
M_COLS = 3 * EXPERT_DIM + NAFF           # 3082
KT = SLICE // 128                        # 4 contraction tiles

FP32 = mybir.dt.float32
F32R = mybir.dt.float32r

# sharding: token shards x column shards (product must be N_CORES)
TOK_SHARDS = 2
COL_SHARDS = 4

USE_F32R = True       # float32r matmuls: 4x faster than fp32 on the PE
VARIANT = "full"      # full | noout | dmaonly  (component profiling)
MCHUNK_TILES = 2      # wall DMA chunk granularity, in 128-col m-tiles
INV_SQRT2 = 0.7071067811865476

_nc_cache: dict = {}


def _col_ranges():
    """Split the M_COLS output columns into COL_SHARDS ranges on 128-multiples."""
    full_tiles = M_COLS // 128            # 24
    per = int(round(full_tiles / COL_SHARDS))
    bounds = [0]
    for s in range(COL_SHARDS - 1):
        bounds.append(min(full_tiles, (s + 1) * per) * 128)
    bounds.append(M_COLS)
    return [(bounds[i], bounds[i + 1]) for i in range(COL_SHARDS)]


def _mchunks(ncols):
    """Group the ncols output columns into DMA chunks of MCHUNK_TILES m-tiles.
    Returns [(c_off, c_cols, [(m_off_in_chunk, mt), ...]), ...]."""
    tiles = []
    m0 = 0
    while m0 < ncols:
        mt = min(128, ncols - m0)
        tiles.append((m0, mt))
        m0 += mt
    chunks = []
    for i in range(0, len(tiles), MCHUNK_TILES):
        grp = tiles[i:i + MCHUNK_TILES]
        c_off = grp[0][0]
        c_cols = sum(mt for _, mt in grp)
        chunks.append((c_off, c_cols, [(m0 - c_off, mt) for (m0, mt) in grp]))
    return chunks


def _prog_key(T, ncols, aw, b2):
    return (T, ncols, aw, b2, USE_F32R, VARIANT, MCHUNK_TILES,
            TOK_SHARDS, COL_SHARDS)


def prepare(inputs):
    """Host-side routing/sharding. Returns (nc, in_maps, meta)."""
    tokens = np.asarray(inputs["tokens"], dtype=np.float32)
    fingerprints = np.asarray(inputs["fingerprints"], dtype=np.float32)
    alpha = np.float32(np.asarray(inputs["alpha"], dtype=np.float32))
    gate_w1 = np.asarray(inputs["gate_w1"], dtype=np.float32)
    gate_b1 = np.asarray(inputs["gate_b1"], dtype=np.float32)
    gate_w2 = np.asarray(inputs["gate_w2"], dtype=np.float32)
    gate_b2 = np.asarray(inputs["gate_b2"], dtype=np.float32)
    wq = np.asarray(inputs["wq"], dtype=np.float32)
    wk = np.asarray(inputs["wk"], dtype=np.float32)
    wv = np.asarray(inputs["wv"], dtype=np.float32)
    penta = np.asarray(inputs["pentachoron"], dtype=np.float32)

    B = tokens.shape[0]
    mask = (fingerprints >= np.float32(FP_MIN)) & (fingerprints < np.float32(FP_MAX))
    idx = np.flatnonzero(mask)
    p = int(idx.size)

    if p == 0:
        return None, None, {"B": B, "p": 0, "mask": mask}

    dirs = penta / np.linalg.norm(penta, axis=-1, keepdims=True)
    dirs = dirs.astype(np.float32)

    # gather + flatten tokens: X [N, 512], feature-major X^T [512, N]
    Xg = tokens[:, idx, S0:S1]                    # [B, p, 512]
    N = B * p
    X2 = Xg.reshape(N, SLICE)
    T = int(math.ceil(N / (TOK_SHARDS * 128))) * 128
    XTfull = np.zeros((SLICE, TOK_SHARDS * T), np.float32)
    XTfull[:, :N] = X2.T

    def pm(a):
        """[512, S] -> partition-major k-interleaved [128, KT*S]."""
        S = a.shape[1]
        return np.ascontiguousarray(
            a.reshape(KT, 128, S).transpose(1, 0, 2).reshape(128, KT * S)
        )

    w1t = pm(gate_w1.T)                            # [128, KT*128]
    b1 = gate_b1.reshape(128, 1)
    bvec = np.ascontiguousarray(
        np.concatenate([b1, b1 * np.float32(INV_SQRT2)], axis=1)
    ).astype(np.float32)
    w2r = np.ascontiguousarray(
        np.repeat(0.5 * gate_w2.reshape(GATE_HID, 1), 128, axis=1)
    ).astype(np.float32)
    aw = float(1.0 / (1.0 + np.exp(-np.float64(alpha))))
    b2 = float(gate_b2.reshape(-1)[0])

    dq = dirs @ wq                                 # [5, 512]
    dk = dirs @ wk
    wall_full = np.concatenate([wq.T, wk.T, wv.T, dq.T, dk.T], axis=1)  # [512, 3082]

    cranges = _col_ranges()
    ncols_prog = max(c1 - c0 for (c0, c1) in cranges)
    mchunks = _mchunks(ncols_prog)

    tchunks = []
    t0 = 0
    while t0 < T:
        tchunks.append((t0, min(512, T - t0)))
        t0 += 512

    in_maps = []
    core_meta = []
    for tshard in range(TOK_SHARDS):
        base = tshard * T
        xs = np.concatenate(
            [pm(XTfull[:, base + t0:base + t0 + tsz]) for (t0, tsz) in tchunks],
            axis=1,
        )
        for (c0, c1) in cranges:
            wall_c = np.zeros((SLICE, ncols_prog), np.float32)
            wall_c[:, : c1 - c0] = wall_full[:, c0:c1]
            # chunk-major wall: for each m-chunk, a k-interleaved block
            blocks = [pm(wall_c[:, co:co + cc]) for (co, cc, _) in mchunks]
            inbuf = np.concatenate([xs, w1t, w2r] + blocks, axis=1)
            in_maps.append({"inbuf": np.ascontiguousarray(inbuf), "bvec": bvec})
            core_meta.append((tshard, c0, c1))

    key = _prog_key(T, ncols_prog, aw, b2)
    if key not in _nc_cache:
        _nc_cache[key] = _build_prog(T, ncols_prog, aw, b2)
    nc = _nc_cache[key]

    meta = {"B": B, "p": p, "N": N, "T": T, "mask": mask, "core_meta": core_meta,
            "prog_args": (T, ncols_prog, aw, b2)}
    return nc, in_maps, meta


def postprocess(results, meta):
    B, p, mask = meta["B"], meta["p"], meta["mask"]
    if p == 0:
        z = np.zeros((B, 0, EXPERT_DIM), np.float32)
        za = np.zeros((5, B, 0), np.float32)
        return (z, z.copy(), z.copy(), za, za.copy(), mask)
    N, T, core_meta = meta["N"], meta["T"], meta["core_meta"]

    O = np.zeros((M_COLS, TOK_SHARDS * T), np.float32)
    for i, (tshard, c0, c1) in enumerate(core_meta):
        O[c0:c1, tshard * T:(tshard + 1) * T] = results[i]["out"][: c1 - c0, :]
    O = O[:, :N]

    D = EXPERT_DIM
    Q = np.ascontiguousarray(O[0:D].T).reshape(B, p, D)
    K = np.ascontiguousarray(O[D:2 * D].T).reshape(B, p, D)
    V = np.ascontiguousarray(O[2 * D:3 * D].T).reshape(B, p, D)
    Q_aff = np.ascontiguousarray(O[3 * D:3 * D + 5]).reshape(5, B, p)
    K_aff = np.ascontiguousarray(O[3 * D + 5:3 * D + 10]).reshape(5, B, p)

    return (Q, K, V, K_aff, Q_aff, mask)


def run(inputs, trace=False):
    """Full pipeline. Returns ((Q, K, V, K_aff, Q_aff, mask), bass_results)."""
    nc, in_maps, meta = prepare(inputs)
    if nc is None:
        return postprocess(None, meta), None
    res = run_bass_kernel_spmd(nc, in_maps, list(range(N_CORES)), trace=trace)
    return postprocess(res.results, meta), res


def _build_prog(T, ncols, aw, b2, reps=1):
    nc = bacc.Bacc()
    built = _build_body(nc, T, ncols, aw, b2, reps)
    built.finalize()
    return built


def _build_body(nc, T, ncols, aw, b2, reps=1):
    mchunks = _mchunks(ncols)
    X_COLS = KT * T
    G_COLS = KT * GATE_HID + 128
    W_COLS = KT * ncols
    ALL = X_COLS + G_COLS + W_COLS
    OFF_X = 0
    OFF_W1 = X_COLS
    OFF_WALL = OFF_W1 + G_COLS
    # per-chunk column offsets within the wall region (chunk-major layout)
    chunk_off = {}
    o = OFF_WALL
    for (co, cc, _) in mchunks:
        chunk_off[co] = o
        o += KT * cc

    MM_DT_IN = F32R if USE_F32R else FP32
    inbuf = nc.dram_tensor("inbuf", [128, ALL], MM_DT_IN, kind="ExternalInput")
    bvec = nc.dram_tensor("bvec", [128, 2], FP32, kind="ExternalInput")
    out = nc.dram_tensor("out", [ncols, T], FP32, kind="ExternalOutput")

    chunks = []
    t0 = 0
    while t0 < T:
        tsz = min(512, T - t0)
        chunks.append((t0, tsz))
        t0 += tsz

    Erf = mybir.ActivationFunctionType.Erf
    Sigmoid = mybir.ActivationFunctionType.Sigmoid
    mul_op = mybir.AluOpType.mult
    add_op = mybir.AluOpType.add
    MM_DT = F32R if USE_F32R else FP32

    with tile.TileContext(nc) as tc:
        with (
            tc.tile_pool(name="weights", bufs=2) as wpool,
            tc.tile_pool(name="gate_sb", bufs=2) as gpool,
            tc.tile_pool(name="gate_ps", bufs=1, space="PSUM") as gpsum,
            tc.tile_pool(name="out_ps", bufs=6, space="PSUM") as opsum,
            tc.tile_pool(name="out_sb", bufs=4) as opool,
        ):
            def rep_ctx():
                if reps < 0:
                    return tc.For_i(0, -reps, 1)
                return contextlib.nullcontext()

            for _rep in range(max(1, reps if reps > 0 else 1)):
                with rep_ctx():
                    # -- input DMAs, emitted in need-order so the scheduler
                    #    prioritizes the critical prefix: gate weights, x0,
                    #    wall0, x1, wall1, ... x on the SP HWDGE ring, weights
                    #    on the SWDGE ring (they stream in parallel) --
                    g_sb = wpool.tile([128, G_COLS], MM_DT, tag="gw")
                    nc.gpsimd.dma_start(g_sb[:], inbuf[:, OFF_W1:OFF_W1 + G_COLS])
                    b_sb = wpool.tile([128, 2], FP32, tag="bvec")
                    nc.sync.dma_start(b_sb[:], bvec[:])
                    x_sb = {}
                    wall_sb = {}
                    xoffs = {}
                    xoff = 0
                    for (t0, tsz) in chunks:
                        xoffs[t0] = xoff
                        xoff += KT * tsz
                    order = []
                    ci = 0
                    mlist = list(mchunks)
                    for i, (t0, tsz) in enumerate(chunks):
                        order.append(("x", (t0, tsz)))
                        take = 1 if i + 1 < len(chunks) else len(mlist) - ci
                        for _ in range(take):
                            if ci < len(mlist):
                                order.append(("w", mlist[ci]))
                                ci += 1
                    while ci < len(mlist):
                        order.append(("w", mlist[ci]))
                        ci += 1
                    for kind, item in order:
                        if kind == "x":
                            (t0, tsz) = item
                            xt_tile = wpool.tile([128, KT * tsz], MM_DT, tag=f"x{t0}")
                            nc.sync.dma_start(
                                xt_tile[:],
                                inbuf[:, OFF_X + xoffs[t0]:OFF_X + xoffs[t0] + KT * tsz],
                            )
                            x_sb[t0] = xt_tile
                        else:
                            (co, cc, _) = item
                            wtile = wpool.tile([128, KT * cc], MM_DT, tag=f"wall{co}")
                            nc.gpsimd.dma_start(
                                wtile[:], inbuf[:, chunk_off[co]:chunk_off[co] + KT * cc]
                            )
                            wall_sb[co] = wtile

                    if VARIANT == "dmaonly":
                        o_sb = opool.tile([128, 8], FP32, tag="o")
                        nc.vector.tensor_copy(o_sb[:, 0:8], x_sb[0][:, 0:8])
                        nc.sync.dma_start(out[0:128, 0:8], o_sb[:, 0:8])
                        continue

                    def xsl(k, t0, tsz):
                        return x_sb[t0][:, k * tsz:k * tsz + tsz]

                    def w1s(k):
                        return g_sb[:, k * GATE_HID:(k + 1) * GATE_HID]

                    w2s = g_sb[:, KT * GATE_HID:KT * GATE_HID + 128]

                    # ---- gate MLP (all token chunks) ----
                    g_gates = {}
                    for (t0, tsz) in chunks:
                        ps_h = gpsum.tile([128, tsz], FP32, tag="ps_h")
                        for k in range(KT):
                            nc.tensor.matmul(
                                ps_h[:], w1s(k), xsl(k, t0, tsz),
                                start=(k == 0), stop=(k == KT - 1),
                            )
                        erf_sb = gpool.tile([128, tsz], FP32, tag="erf")
                        nc.scalar.activation(
                            erf_sb[:], ps_h[:], Erf,
                            bias=b_sb[:, 1:2], scale=INV_SQRT2,
                        )
                        xb_sb = gpool.tile([128, tsz], FP32, tag="xb")
                        nc.vector.tensor_scalar_add(xb_sb[:], ps_h[:], b_sb[:, 0:1])
                        # h2 = (erf + 1) * xb  (= 2*gelu(x); 0.5 folded into w2r)
                        h2_sb = gpool.tile([128, tsz], MM_DT, tag="h2")
                        nc.vector.scalar_tensor_tensor(
                            h2_sb[:], erf_sb[:], 1.0, xb_sb[:], add_op, mul_op
                        )
                        ps_g = gpsum.tile([128, tsz], FP32, tag="ps_g")
                        nc.tensor.matmul(ps_g[:], w2s, h2_sb[:], start=True, stop=True)
                        sig_sb = gpool.tile([128, tsz], FP32, tag="sig")
                        nc.scalar.activation(sig_sb[:], ps_g[:], Sigmoid, bias=b2)
                        g_gate = gpool.tile([128, tsz], FP32, tag=f"g{t0}")
                        nc.vector.tensor_scalar(
                            g_gate[:], sig_sb[:], aw, 1.0 - aw, mul_op, add_op
                        )
                        g_gates[t0] = g_gate

                    # ---- QKV + folded affinities: m-outer, t-inner,
                    #      one combined output DMA per m-tile ----
                    for (co, cc, tiles) in mchunks:
                        wtile = wall_sb[co]
                        for (mo, mt) in tiles:
                            o_sb = opool.tile([128, T], FP32, tag="o")
                            for (t0, tsz) in chunks:
                                ps_o = opsum.tile([128, tsz], FP32, tag="ps_o")
                                for k in range(KT):
                                    nc.tensor.matmul(
                                        ps_o[:mt, :],
                                        wtile[:, k * cc + mo:k * cc + mo + mt],
                                        xsl(k, t0, tsz),
                                        start=(k == 0), stop=(k == KT - 1),
                                    )
                                nc.vector.tensor_mul(
                                    o_sb[:mt, t0:t0 + tsz], ps_o[:mt, :],
                                    g_gates[t0][:mt, :],
                                )
                            if VARIANT != "noout":
                                nc.scalar.dma_start(
                                    out[co + mo:co + mo + mt, :], o_sb[:mt, :]
                                )
    return nc


def kernel(**inputs):
    outputs, _ = run(inputs, trace=False)
    return outputs
